# revision 28
# baseline (speedup 1.0000x reference)
"""VQ codebook (Memory Block) Trainium2 kernel — v2.

Data-parallel over n = b*h*w across 8 NeuronCores (core i takes batch b=i,
whose x[i] slice in (B,C,H,W) layout is already the (c, n_loc) transposed
operand the matmuls need). The EMA segment-sum update is computed as a dense
one-hot GEMM per core and AllReduced across cores.

v2: fp16 matmuls everywhere (2x PE throughput vs fp32 LOW_HIGH), x pre-cast
to fp16 on host, DMA-xbar transposes instead of PE transposes, softmax Z via
activation accum_out.
"""

from contextlib import ExitStack

import numpy as np

import concourse.bass as bass
import concourse.bacc as bacc
import concourse.tile as tile
import concourse.mybir as mybir
from concourse.bass_utils import run_bass_kernel_spmd

B, C, H, W = 8, 512, 64, 64
K, V = 1024, 512
NCORES = 8
NLOC = B * H * W // NCORES      # 4096 rows per core
NT = NLOC // 128                # 32 subtiles of 128 rows
RATE = 0.999

f32 = mybir.dt.float32
f16 = mybir.dt.float16
AX = mybir.AxisListType.X
AF = mybir.ActivationFunctionType
ALU = mybir.AluOpType


def _normalize_rows(nc, pool, src, rs_tag):
    """src: (128, C) f32 tile -> returns normalized tile (128, C) f32."""
    sq = pool.tile([128, C], f32, tag=rs_tag + "sq", name=rs_tag + "sq")
    rs = pool.tile([128, 1], f32, tag=rs_tag + "rs", name=rs_tag + "rs")
    nc.scalar.activation(sq[:], src[:], AF.Square, accum_out=rs[:])
    nrm = pool.tile([128, 1], f32, tag=rs_tag + "nrm", name=rs_tag + "nrm")
    nc.scalar.activation(nrm[:], rs[:], AF.Sqrt)
    nrm2 = pool.tile([128, 1], f32, tag=rs_tag + "nrm2", name=rs_tag + "nrm2")
    nc.vector.tensor_scalar_max(nrm2[:], nrm[:], 1e-12)
    rin = pool.tile([128, 1], f32, tag=rs_tag + "rin", name=rs_tag + "rin")
    nc.vector.reciprocal(rin[:], nrm2[:])
    out = pool.tile([128, C], f32, tag=rs_tag + "out", name=rs_tag + "out")
    nc.vector.tensor_scalar_mul(out[:], src[:], rin[:])
    return out


def _kernel(ctx, tc, x16, m, stdt, ident_d, score2_o, outv_o):
    nc = tc.nc

    const = ctx.enter_context(tc.tile_pool(name="const", bufs=1))
    ones_h = const.tile([128, 1], f16)
    nc.vector.memset(ones_h[:], 1.0)
    ident = const.tile([128, 128], f32)
    nc.sync.dma_start(ident[:], ident_d[:])
    ident16 = const.tile([128, 128], f16)
    nc.vector.tensor_copy(ident16[:], ident[:])

    # persistent across phases
    big = ctx.enter_context(tc.tile_pool(name="big", bufs=1))
    norms = big.tile([128, NT], f32)          # 1/||xf_row||, col t
    cnt_acc = big.tile([128, K], f32)         # per-partition onehot sums
    embed_sb = big.tile([128, 8 * 512], f16)  # embed_sum, chunk j at cols j*512
    counts_sb = big.tile([128, 8], f16)
    std16 = big.tile([128, 8 * 512], f16)     # std chunk j at cols j*512
    xb16 = [big.tile([128, NLOC], f16, name=f"xb16_{cc}") for cc in range(4)]
    mT_pool = ctx.enter_context(tc.tile_pool(name="mT", bufs=1))
    mnT = [mT_pool.tile([128, K], f16, tag=f"mnT{cc}", name=f"mnT{cc}")
           for cc in range(4)]
    mn2T = [mT_pool.tile([128, K], f16, tag=f"mn2T{cc}", name=f"mn2T{cc}")
            for cc in range(4)]

    # transpose psum pool shared by all phases
    psT = ctx.enter_context(tc.tile_pool(name="psT", bufs=2, space="PSUM"))


    # ncfw warmup: tiny AllReduce so the real ones skip cold-start costs
    dram0 = ctx.enter_context(tc.tile_pool(name="dram0", bufs=1, space="DRAM"))
    ccw_i = dram0.tile([1, 512], f16)
    ccw_o = dram0.tile([1, 512], f16, addr_space="Shared")
    warm_sb = const.tile([1, 512], f16)
    nc.vector.memset(warm_sb[:], 0.0)
    nc.gpsimd.dma_start(ccw_i[:], warm_sb[:])
    nc.gpsimd.collective_compute(
        "AllReduce", ALU.add, replica_groups=[list(range(NCORES))],
        ins=[ccw_i.opt()], outs=[ccw_o.opt()])

    # ---- prologue: mnT = normalize(m).T ----
    with tc.tile_pool(name="prol", bufs=3) as prol:
        mjs = []
        for j in range(8):
            mj = prol.tile([128, C], f32, tag=f"mj{j}", name=f"mj{j}")
            nc.sync.dma_start(mj[:], m[j * 128:(j + 1) * 128, :])
            mjs.append(mj)
        # x (c-part fp16) resident for phases A and C
        for cc in range(4):
            nc.gpsimd.dma_start(xb16[cc][:], x16[cc * 128:(cc + 1) * 128, :])
        for j in range(8):
            mj = mjs[j]
            mnj = _normalize_rows(nc, prol, mj, "p")
            for cc in range(4):
                pt = psT.tile([128, 128], f32, tag="pt", name="pt")
                nc.tensor.transpose(pt[:], mnj[:, cc * 128:(cc + 1) * 128], ident[:])
                nc.vector.tensor_copy(mnT[cc][:, j * 128:(j + 1) * 128], pt[:])

    # HAM warmup: chained matmuls fill the wait for mnT (prologue chain)
    # so the PE clock gate is open when phase A starts
    with tc.tile_pool(name="warm", bufs=1, space="PSUM") as warm:
        wps = warm.tile([128, 512], f32)
        wrhs = const.tile([128, 512], f16)
        nc.vector.memset(wrhs[:], 0.0)
        for i in range(32):
            nc.tensor.matmul(wps[:], ident16[:], wrhs[:],
                             start=(i == 0), stop=(i == 31))

    # ---- phase A ----
    ab_ctx = ExitStack()
    ab = ab_ctx.enter_context(tc.tile_pool(name="ab", bufs=1))
    onehot = ab.tile([128, NT * K], f16)    # subtile t at cols t*K
    xfT = ab.tile([128, NT * C], f16)       # subtile t at cols t*C
    psE_ctx = ExitStack()
    psE = psE_ctx.enter_context(tc.tile_pool(name="psE", bufs=4, space="PSUM"))
    eps = [psE.tile([128, 512], f32, tag="eb", name="eb") for _ in range(4)]
    nc.vector.memset(cnt_acc[:], 0.0)
    with (
        tc.tile_pool(name="sqp", bufs=3) as sqp,
        tc.tile_pool(name="psA", bufs=2, space="PSUM") as psA,
        tc.tile_pool(name="wa", bufs=3) as wa,
    ):
        def embed_k0(t):
            for jj in range(4):
                oh = onehot[:, t * K + jj * 128: t * K + (jj + 1) * 128]
                nc.tensor.matmul(eps[jj][:], oh, xfT[:, t * C:(t + 1) * C],
                                 start=(t == 0), stop=(t == NT - 1))
        for t in range(NT):
            off = t * 128
            for cc in range(4):
                pt16 = psT.tile([128, 128], f16, tag="pt", name="pt16")
                nc.tensor.transpose(pt16[:], xb16[cc][:, off:off + 128], ident16[:])
                nc.vector.tensor_copy(
                    xfT[:, t * C + cc * 128: t * C + (cc + 1) * 128], pt16[:])
            # row sum-of-squares from the n-part layout via ACT accumulator
            sq = sqp.tile([128, 512], f16, tag="sq", name="sq")
            ssq = wa.tile([128, 1], f32, tag="ssq", name="ssq")
            nc.scalar.activation(sq[:], xfT[:, t * C:(t + 1) * C], AF.Square,
                                 accum_out=ssq[:])
            nrm = wa.tile([128, 1], f32, tag="nrm", name="nrm")
            nc.scalar.activation(nrm[:], ssq[:], AF.Sqrt)
            nrm2 = wa.tile([128, 1], f32, tag="nrm2", name="nrm2")
            nc.vector.tensor_scalar_max(nrm2[:], nrm[:], 1e-12)
            nc.vector.reciprocal(norms[:, t:t + 1], nrm2[:])
            ps1 = [psA.tile([128, 512], f32, tag="raw1", name="raw1")
                   for _ in range(2)]
            for cc in range(4):
                for kb in range(2):
                    nc.tensor.matmul(
                        ps1[kb][:], xb16[cc][:, off:off + 128],
                        mnT[cc][:, kb * 512:(kb + 1) * 512],
                        start=(cc == 0), stop=(cc == 3))
            if t > 0:
                embed_k0(t - 1)   # one subtile behind: its onehot is ready
            rmax2 = wa.tile([128, 2], f32, tag="rmax2", name="rmax2")
            for kb in range(2):
                nc.vector.reduce_max(rmax2[:, kb:kb + 1], ps1[kb][:], axis=AX)
            rmax = wa.tile([128, 1], f32, tag="rmax", name="rmax")
            nc.vector.reduce_max(rmax[:], rmax2[:], axis=AX)
            for kb in range(2):
                nc.vector.tensor_scalar(
                    onehot[:, t * K + kb * 512: t * K + (kb + 1) * 512],
                    ps1[kb][:], rmax[:], None, ALU.is_equal)
            nc.gpsimd.tensor_add(cnt_acc[:], cnt_acc[:],
                                  onehot[:, t * K:(t + 1) * K])
        embed_k0(NT - 1)

    # ---- ship k-half 0 + counts (AllReduce 1), then k-half 1 GEMM ----
    dram = ctx.enter_context(tc.tile_pool(name="dram", bufs=1, space="DRAM"))
    ccin = [dram.tile([514 - 2 * h, 512], f16, name=f"ccin{h}") for h in range(2)]
    ccout = [dram.tile([514 - 2 * h, 512], f16, addr_space="Shared",
                       name=f"ccout{h}") for h in range(2)]
    ones_f = const.tile([128, 1], f32)
    nc.vector.memset(ones_f[:], 1.0)
    with tc.tile_pool(name="psR", bufs=1, space="PSUM") as psR:
        for jj in range(4):
            nc.vector.tensor_copy(embed_sb[:, jj * 512:(jj + 1) * 512], eps[jj][:])
        # counts: partition-reduce cnt_acc -> (1, K) -> transpose to (128, 8)
        crow = psR.tile([1, K], f32)
        for kb in range(2):
            nc.tensor.matmul(crow[0:1, kb * 512:(kb + 1) * 512], ones_f[:],
                             cnt_acc[:, kb * 512:(kb + 1) * 512],
                             start=True, stop=True)
        crow_sb = const.tile([1, K], f16)
        nc.vector.tensor_copy(crow_sb[:], crow[:])
        for jj in range(4):
            nc.gpsimd.dma_start(ccin[0][jj * 128:(jj + 1) * 128, :],
                                embed_sb[:, jj * 512:(jj + 1) * 512])
        nc.gpsimd.dma_start(
            ccin[0][512:514, :].rearrange("a b -> (a b)").rearrange(
                "(a b) -> a b", a=1),
            crow_sb[:])
        nc.gpsimd.collective_compute(
            "AllReduce", ALU.add, replica_groups=[list(range(NCORES))],
            ins=[ccin[0].opt()], outs=[ccout[0].opt()])
        # k-half 1 GEMM while AllReduce 1 is in flight
        eps2 = [psE.tile([128, 512], f32, tag="eb", name="eb2") for _ in range(4)]
        for t in range(NT):
            for jj in range(4):
                j = 4 + jj
                oh = onehot[:, t * K + j * 128: t * K + (j + 1) * 128]
                nc.tensor.matmul(eps2[jj][:], oh, xfT[:, t * C:(t + 1) * C],
                                 start=(t == 0), stop=(t == NT - 1))
        for jj in range(4):
            j = 4 + jj
            nc.vector.tensor_copy(embed_sb[:, j * 512:(j + 1) * 512], eps2[jj][:])
            nc.gpsimd.dma_start(ccin[1][jj * 128:(jj + 1) * 128, :],
                                embed_sb[:, j * 512:(j + 1) * 512])
        nc.gpsimd.collective_compute(
            "AllReduce", ALU.add, replica_groups=[list(range(NCORES))],
            ins=[ccin[1].opt()], outs=[ccout[1].opt()])
    psE_ctx.close()  # release embed PSUM banks before phase C
    ab_ctx.close()  # release onehot/xfT SBUF before phase C

    # load std during the collective window (fp16 via DVE cast)
    with tc.tile_pool(name="stdl", bufs=2) as stdl:
        for j in range(8):
            sj = stdl.tile([128, 512], f32, tag="sj", name="sj")
            nc.gpsimd.dma_start(sj[:], stdt[j * 128:(j + 1) * 128, :])
            nc.vector.tensor_copy(std16[:, j * 512:(j + 1) * 512], sj[:])

    # ---- EMA update -> mn2T (per collective half) + phase-C head-start ----
    HS = 16
    s2h_pool = ctx.enter_context(tc.tile_pool(name="s2h", bufs=HS))
    s2h = []
    psC = ctx.enter_context(tc.tile_pool(name="psC", bufs=4, space="PSUM"))
    psU = ctx.enter_context(tc.tile_pool(name="psU", bufs=2, space="PSUM"))
    with tc.tile_pool(name="ema", bufs=2) as ema:
        for half in range(2):
            counts_g = ema.tile([128, 4], f16, tag="cg", name="cg")
            cc_cnt = ccout[0][512:514, :].rearrange("a b -> (a b)").rearrange(
                "(j p) -> p j", p=128)
            nc.gpsimd.dma_start(counts_g[:], cc_cnt[:, half * 4:(half + 1) * 4])
            ceps = ema.tile([128, 4], f32, tag="ce", name="ce")
            nc.vector.tensor_scalar_add(ceps[:], counts_g[:], 1e-6)
            crec = ema.tile([128, 4], f32, tag="cr", name="cr")
            nc.vector.reciprocal(crec[:], ceps[:])
            for jj in range(4):
                j = half * 4 + jj
                eg = ema.tile([128, 512], f16, tag="eg", name="eg")
                nc.gpsimd.dma_start(eg[:], ccout[half][jj * 128:(jj + 1) * 128, :])
                mj = ema.tile([128, 512], f32, tag="mj2", name="mj2")
                nc.gpsimd.dma_start(mj[:], m[j * 128:(j + 1) * 128, :])
                em = ema.tile([128, 512], f32, tag="em", name="em")
                nc.vector.tensor_scalar(em[:], eg[:], crec[:, jj:jj + 1], 1.0 - RATE,
                                        ALU.mult, op1=ALU.mult)
                nm = ema.tile([128, 512], f32, tag="nm", name="nm")
                nc.vector.scalar_tensor_tensor(nm[:], mj[:], RATE, em[:],
                                               ALU.mult, ALU.add)
                mn2j = _normalize_rows(nc, ema, nm, "e")
                for cc in range(4):
                    pt = psT.tile([128, 128], f32, tag="pt", name="pt")
                    nc.tensor.transpose(pt[:], mn2j[:, cc * 128:(cc + 1) * 128],
                                        ident[:])
                    nc.vector.tensor_copy(mn2T[cc][:, j * 128:(j + 1) * 128], pt[:])
            if half == 0:
                # head-start: first HS subtiles' k-lower raw2 while the second
                # collective half is still in flight
                for t in range(HS):
                    off = t * 128
                    ph = psC.tile([128, 512], f32, tag="raw2", name="raw2")
                    for cc in range(4):
                        nc.tensor.matmul(ph[:], xb16[cc][:, off:off + 128],
                                         mn2T[cc][:, 0:512],
                                         start=(cc == 0), stop=(cc == 3))
                    sh = s2h_pool.tile([128, 512], f32, tag="s2h", name=f"s2h{t}")
                    nc.vector.tensor_scalar_mul(sh[:], ph[:], norms[:, t:t + 1])
                    s2h.append(sh)

    # ---- phase C: score2, softmax, out ----
    with tc.tile_pool(name="wc", bufs=3) as wc:
        for t in range(NT):
            off = t * 128
            halves = []
            for kb in range(2):
                if kb == 0 and t < HS:
                    halves.append(s2h[t])
                    continue
                p = psC.tile([128, 512], f32, tag="raw2", name="raw2")
                for cc in range(4):
                    nc.tensor.matmul(
                        p[:], xb16[cc][:, off:off + 128],
                        mn2T[cc][:, kb * 512:(kb + 1) * 512],
                        start=(cc == 0), stop=(cc == 3))
                sv = wc.tile([128, 512], f32, tag=f"s2{kb}", name=f"s2{kb}")
                nc.vector.tensor_scalar_mul(sv[:], p[:], norms[:, t:t + 1])
                halves.append(sv)
            e_sb = wc.tile([128, K], f32, tag="esb", name="esb")
            zk = wc.tile([128, 2], f32, tag="zk", name="zk")
            for kb in range(2):
                nc.gpsimd.dma_start(
                    score2_o[t * 128:(t + 1) * 128, kb * 512:(kb + 1) * 512],
                    halves[kb][:])
                nc.scalar.activation(
                    e_sb[:, kb * 512:(kb + 1) * 512], halves[kb][:], AF.Exp,
                    accum_out=zk[:, kb:kb + 1])
            z = wc.tile([128, 1], f32, tag="z", name="z")
            nc.vector.tensor_add(z[:], zk[:, 0:1], zk[:, 1:2])
            zr = wc.tile([128, 1], f32, tag="zr", name="zr")
            nc.vector.reciprocal(zr[:], z[:])
            et = wc.tile([128, K], f16, tag="et", name="et")
            for j in range(8):
                pt = psT.tile([128, 128], f32, tag="pt", name="pt")
                nc.tensor.transpose(pt[:], e_sb[:, j * 128:(j + 1) * 128], ident[:])
                nc.vector.tensor_copy(et[:, j * 128:(j + 1) * 128], pt[:])
            pu = psU.tile([128, 512], f32, tag="pu", name="pu")
            for j in range(8):
                nc.tensor.matmul(pu[:], et[:, j * 128:(j + 1) * 128],
                                 std16[:, j * 512:(j + 1) * 512],
                                 start=(j == 0), stop=(j == 7))
            uo = wc.tile([128, 512], f32, tag="uo", name="uo")
            nc.vector.tensor_scalar_mul(uo[:], pu[:], zr[:])
            nc.gpsimd.dma_start(outv_o[t * 128:(t + 1) * 128, :], uo[:])


_NC_CACHE = None


def _build():
    global _NC_CACHE
    if _NC_CACHE is not None:
        return _NC_CACHE
    nc = bacc.Bacc("TRN2", target_bir_lowering=False, debug=False,
                   enable_asserts=True, num_devices=NCORES)
    x16 = nc.dram_tensor("x16", [C, NLOC], f16, kind="ExternalInput").ap()
    m = nc.dram_tensor("m", [K, C], f32, kind="ExternalInput").ap()
    stdt = nc.dram_tensor("stdt", [K, V], f32, kind="ExternalInput").ap()
    ident = nc.dram_tensor("ident", [128, 128], f32, kind="ExternalInput").ap()
    score2_o = nc.dram_tensor("score2", [NLOC, K], f32, kind="ExternalOutput").ap()
    outv_o = nc.dram_tensor("outv", [NLOC, V], f32, kind="ExternalOutput").ap()
    with tile.TileContext(nc, trace_sim=False) as tc, ExitStack() as ctx:
        _kernel(ctx, tc, x16, m, stdt, ident, score2_o, outv_o)
    nc.compile()
    _NC_CACHE = nc
    return nc


def _in_maps(x, noise_feature, std):
    return [
        {
            "x16": np.ascontiguousarray(
                x[i].reshape(C, NLOC)).astype(np.float16),
            "m": np.asarray(noise_feature, dtype=np.float32),
            "stdt": np.asarray(std, dtype=np.float32),
            "ident": np.eye(128, dtype=np.float32),
        }
        for i in range(NCORES)
    ]


def _gather(results):
    score2 = np.concatenate([r["score2"] for r in results], axis=0)
    out = np.stack([r["outv"].reshape(H, W, V) for r in results], axis=0)
    return out, score2


def kernel(x, noise_feature, std):
    nc = _build()
    res = run_bass_kernel_spmd(
        nc, _in_maps(np.asarray(x), np.asarray(noise_feature), np.asarray(std)),
        core_ids=list(range(NCORES)))
    return _gather(res.results)


# revision 29
# speedup vs baseline: 1.0107x; 1.0107x over previous
"""VQ codebook (Memory Block) Trainium2 kernel.

Data-parallel over n = b*h*w across 8 NeuronCores: core i takes batch b=i,
whose x[i] slice in (B,C,H,W) layout is already the (c, n_loc) transposed
operand every matmul needs. Per core: scores vs the normalized codebook
(fp16 PE matmuls), argmax via is_equal(row-max), the EMA segment-sum as a
dense one-hot GEMM (k-half 0 interleaved into the score pass one subtile
behind the argmax, k-half 1 as its own pass under the first AllReduce),
counts accumulated on GpSimd + a ones-matmul partition reduction. The
(embed_sum || counts) AllReduce is split in two fp16 halves pipelined
against the k-half-1 GEMM and a phase-C head-start; softmax skips the
max-shift (score2 is a cosine similarity, |s| <= 1) and gets Z from the
ACT accumulator; E is PE-transposed for the final E @ std matmul.
"""

from contextlib import ExitStack

import numpy as np

import concourse.bass as bass
import concourse.bacc as bacc
import concourse.tile as tile
import concourse.mybir as mybir
from concourse.bass_utils import run_bass_kernel_spmd

B, C, H, W = 8, 512, 64, 64
K, V = 1024, 512
NCORES = 8
NLOC = B * H * W // NCORES      # 4096 rows per core
NT = NLOC // 128                # 32 subtiles of 128 rows
RATE = 0.999

f32 = mybir.dt.float32
f16 = mybir.dt.float16
AX = mybir.AxisListType.X
AF = mybir.ActivationFunctionType
ALU = mybir.AluOpType


def _normalize_rows(nc, pool, src, rs_tag):
    """src: (128, C) f32 tile -> returns normalized tile (128, C) f32."""
    sq = pool.tile([128, C], f32, tag=rs_tag + "sq", name=rs_tag + "sq")
    rs = pool.tile([128, 1], f32, tag=rs_tag + "rs", name=rs_tag + "rs")
    nc.scalar.activation(sq[:], src[:], AF.Square, accum_out=rs[:])
    nrm = pool.tile([128, 1], f32, tag=rs_tag + "nrm", name=rs_tag + "nrm")
    nc.scalar.activation(nrm[:], rs[:], AF.Sqrt)
    nrm2 = pool.tile([128, 1], f32, tag=rs_tag + "nrm2", name=rs_tag + "nrm2")
    nc.vector.tensor_scalar_max(nrm2[:], nrm[:], 1e-12)
    rin = pool.tile([128, 1], f32, tag=rs_tag + "rin", name=rs_tag + "rin")
    nc.vector.reciprocal(rin[:], nrm2[:])
    out = pool.tile([128, C], f32, tag=rs_tag + "out", name=rs_tag + "out")
    nc.vector.tensor_scalar_mul(out[:], src[:], rin[:])
    return out


def _kernel(ctx, tc, x16, m, stdt, ident_d, score2_o, outv_o):
    nc = tc.nc

    const = ctx.enter_context(tc.tile_pool(name="const", bufs=1))
    ones_h = const.tile([128, 1], f16)
    nc.vector.memset(ones_h[:], 1.0)
    ident = const.tile([128, 128], f32)
    nc.sync.dma_start(ident[:], ident_d[:])
    ident16 = const.tile([128, 128], f16)
    nc.vector.tensor_copy(ident16[:], ident[:])

    # persistent across phases
    big = ctx.enter_context(tc.tile_pool(name="big", bufs=1))
    norms = big.tile([128, NT], f32)          # 1/||xf_row||, col t
    cnt_acc = big.tile([128, K], f32)         # per-partition onehot sums
    embed_sb = big.tile([128, 8 * 512], f16)  # embed_sum, chunk j at cols j*512
    counts_sb = big.tile([128, 8], f16)
    std16 = big.tile([128, 8 * 512], f16)     # std chunk j at cols j*512
    xb16 = [big.tile([128, NLOC], f16, name=f"xb16_{cc}") for cc in range(4)]
    mT_pool = ctx.enter_context(tc.tile_pool(name="mT", bufs=1))
    mnT = [mT_pool.tile([128, K], f16, tag=f"mnT{cc}", name=f"mnT{cc}")
           for cc in range(4)]
    mn2T = [mT_pool.tile([128, K], f16, tag=f"mn2T{cc}", name=f"mn2T{cc}")
            for cc in range(4)]

    # transpose psum pool shared by all phases
    psT = ctx.enter_context(tc.tile_pool(name="psT", bufs=2, space="PSUM"))


    # ncfw warmup: tiny AllReduce so the real ones skip cold-start costs
    dram0 = ctx.enter_context(tc.tile_pool(name="dram0", bufs=1, space="DRAM"))
    ccw_i = dram0.tile([1, 512], f16)
    ccw_o = dram0.tile([1, 512], f16, addr_space="Shared")
    warm_sb = const.tile([1, 512], f16)
    nc.vector.memset(warm_sb[:], 0.0)
    nc.gpsimd.dma_start(ccw_i[:], warm_sb[:])
    nc.gpsimd.collective_compute(
        "AllReduce", ALU.add, replica_groups=[list(range(NCORES))],
        ins=[ccw_i.opt()], outs=[ccw_o.opt()])

    # ---- prologue: mnT = normalize(m).T ----
    with tc.tile_pool(name="prol", bufs=3) as prol:
        mjs = []
        for j in range(8):
            mj = prol.tile([128, C], f32, tag=f"mj{j}", name=f"mj{j}")
            nc.sync.dma_start(mj[:], m[j * 128:(j + 1) * 128, :])
            mjs.append(mj)
        # x (c-part fp16) resident for phases A and C
        for cc in range(4):
            nc.gpsimd.dma_start(xb16[cc][:], x16[cc * 128:(cc + 1) * 128, :])
        for j in range(8):
            mj = mjs[j]
            mnj = _normalize_rows(nc, prol, mj, "p")
            for cc in range(4):
                pt = psT.tile([128, 128], f32, tag="pt", name="pt")
                nc.tensor.transpose(pt[:], mnj[:, cc * 128:(cc + 1) * 128], ident[:])
                nc.vector.tensor_copy(mnT[cc][:, j * 128:(j + 1) * 128], pt[:])

    # HAM warmup: chained matmuls fill the wait for mnT (prologue chain)
    # so the PE clock gate is open when phase A starts
    with tc.tile_pool(name="warm", bufs=1, space="PSUM") as warm:
        wps = warm.tile([128, 512], f32)
        wrhs = const.tile([128, 512], f16)
        nc.vector.memset(wrhs[:], 0.0)
        for i in range(32):
            nc.tensor.matmul(wps[:], ident16[:], wrhs[:],
                             start=(i == 0), stop=(i == 31))

    # ---- phase A ----
    ab_ctx = ExitStack()
    ab = ab_ctx.enter_context(tc.tile_pool(name="ab", bufs=1))
    onehot = ab.tile([128, NT * K], f16)    # subtile t at cols t*K
    xfT = ab.tile([128, NT * C], f16)       # subtile t at cols t*C
    psE_ctx = ExitStack()
    psE = psE_ctx.enter_context(tc.tile_pool(name="psE", bufs=4, space="PSUM"))
    eps = [psE.tile([128, 512], f32, tag="eb", name="eb") for _ in range(4)]
    nc.vector.memset(cnt_acc[:], 0.0)
    with (
        tc.tile_pool(name="sqp", bufs=3) as sqp,
        tc.tile_pool(name="psA", bufs=2, space="PSUM") as psA,
        tc.tile_pool(name="wa", bufs=3) as wa,
    ):
        def embed_k0(t):
            for jj in range(4):
                oh = onehot[:, t * K + jj * 128: t * K + (jj + 1) * 128]
                nc.tensor.matmul(eps[jj][:], oh, xfT[:, t * C:(t + 1) * C],
                                 start=(t == 0), stop=(t == NT - 1))
        for t in range(NT):
            off = t * 128
            for cc in range(4):
                pt16 = psT.tile([128, 128], f16, tag="pt", name="pt16")
                nc.tensor.transpose(pt16[:], xb16[cc][:, off:off + 128], ident16[:])
                nc.vector.tensor_copy(
                    xfT[:, t * C + cc * 128: t * C + (cc + 1) * 128], pt16[:])
            # row sum-of-squares from the n-part layout via ACT accumulator
            sq = sqp.tile([128, 512], f16, tag="sq", name="sq")
            ssq = wa.tile([128, 1], f32, tag="ssq", name="ssq")
            nc.scalar.activation(sq[:], xfT[:, t * C:(t + 1) * C], AF.Square,
                                 accum_out=ssq[:])
            nrm = wa.tile([128, 1], f32, tag="nrm", name="nrm")
            nc.scalar.activation(nrm[:], ssq[:], AF.Sqrt)
            nrm2 = wa.tile([128, 1], f32, tag="nrm2", name="nrm2")
            nc.vector.tensor_scalar_max(nrm2[:], nrm[:], 1e-12)
            nc.vector.reciprocal(norms[:, t:t + 1], nrm2[:])
            ps1 = [psA.tile([128, 512], f32, tag="raw1", name="raw1")
                   for _ in range(2)]
            for cc in range(4):
                for kb in range(2):
                    nc.tensor.matmul(
                        ps1[kb][:], xb16[cc][:, off:off + 128],
                        mnT[cc][:, kb * 512:(kb + 1) * 512],
                        start=(cc == 0), stop=(cc == 3))
            if t > 0:
                embed_k0(t - 1)   # one subtile behind: its onehot is ready
            rmax2 = wa.tile([128, 2], f32, tag="rmax2", name="rmax2")
            for kb in range(2):
                nc.vector.reduce_max(rmax2[:, kb:kb + 1], ps1[kb][:], axis=AX)
            rmax = wa.tile([128, 1], f32, tag="rmax", name="rmax")
            nc.vector.reduce_max(rmax[:], rmax2[:], axis=AX)
            for kb in range(2):
                nc.vector.tensor_scalar(
                    onehot[:, t * K + kb * 512: t * K + (kb + 1) * 512],
                    ps1[kb][:], rmax[:], None, ALU.is_equal)
            nc.gpsimd.tensor_add(cnt_acc[:], cnt_acc[:],
                                  onehot[:, t * K:(t + 1) * K])
        embed_k0(NT - 1)

    # ---- ship k-half 0 + counts (AllReduce 1), then k-half 1 GEMM ----
    dram = ctx.enter_context(tc.tile_pool(name="dram", bufs=1, space="DRAM"))
    ccin = [dram.tile([514 - 2 * h, 512], f16, name=f"ccin{h}") for h in range(2)]
    ccout = [dram.tile([514 - 2 * h, 512], f16, addr_space="Shared",
                       name=f"ccout{h}") for h in range(2)]
    ones_f = const.tile([128, 1], f32)
    nc.vector.memset(ones_f[:], 1.0)
    with tc.tile_pool(name="psR", bufs=1, space="PSUM") as psR:
        for jj in range(4):
            nc.vector.tensor_copy(embed_sb[:, jj * 512:(jj + 1) * 512], eps[jj][:])
        # counts: partition-reduce cnt_acc -> (1, K) -> transpose to (128, 8)
        crow = psR.tile([1, K], f32)
        for kb in range(2):
            nc.tensor.matmul(crow[0:1, kb * 512:(kb + 1) * 512], ones_f[:],
                             cnt_acc[:, kb * 512:(kb + 1) * 512],
                             start=True, stop=True)
        crow_sb = const.tile([1, K], f16)
        nc.vector.tensor_copy(crow_sb[:], crow[:])
        for jj in range(4):
            nc.gpsimd.dma_start(ccin[0][jj * 128:(jj + 1) * 128, :],
                                embed_sb[:, jj * 512:(jj + 1) * 512])
        nc.gpsimd.dma_start(
            ccin[0][512:514, :].rearrange("a b -> (a b)").rearrange(
                "(a b) -> a b", a=1),
            crow_sb[:])
        nc.gpsimd.collective_compute(
            "AllReduce", ALU.add, replica_groups=[list(range(NCORES))],
            ins=[ccin[0].opt()], outs=[ccout[0].opt()])
        # k-half 1 GEMM while AllReduce 1 is in flight
        eps2 = [psE.tile([128, 512], f32, tag="eb", name="eb2") for _ in range(4)]
        for t in range(NT):
            for jj in range(4):
                j = 4 + jj
                oh = onehot[:, t * K + j * 128: t * K + (j + 1) * 128]
                nc.tensor.matmul(eps2[jj][:], oh, xfT[:, t * C:(t + 1) * C],
                                 start=(t == 0), stop=(t == NT - 1))
        for jj in range(4):
            j = 4 + jj
            nc.vector.tensor_copy(embed_sb[:, j * 512:(j + 1) * 512], eps2[jj][:])
            nc.gpsimd.dma_start(ccin[1][jj * 128:(jj + 1) * 128, :],
                                embed_sb[:, j * 512:(j + 1) * 512])
        nc.gpsimd.collective_compute(
            "AllReduce", ALU.add, replica_groups=[list(range(NCORES))],
            ins=[ccin[1].opt()], outs=[ccout[1].opt()])
    psE_ctx.close()  # release embed PSUM banks before phase C
    ab_ctx.close()  # release onehot/xfT SBUF before phase C

    # load std during the collective window (fp16 via DVE cast)
    with tc.tile_pool(name="stdl", bufs=2) as stdl:
        for j in range(8):
            sj = stdl.tile([128, 512], f32, tag="sj", name="sj")
            nc.gpsimd.dma_start(sj[:], stdt[j * 128:(j + 1) * 128, :])
            nc.vector.tensor_copy(std16[:, j * 512:(j + 1) * 512], sj[:])

    # ---- EMA update -> mn2T (per collective half) + phase-C head-start ----
    HS = 16
    s2h_pool = ctx.enter_context(tc.tile_pool(name="s2h", bufs=HS))
    s2h = []
    psC = ctx.enter_context(tc.tile_pool(name="psC", bufs=4, space="PSUM"))
    psU = ctx.enter_context(tc.tile_pool(name="psU", bufs=2, space="PSUM"))
    with tc.tile_pool(name="ema", bufs=2) as ema:
        for half in range(2):
            counts_g = ema.tile([128, 4], f16, tag="cg", name="cg")
            cc_cnt = ccout[0][512:514, :].rearrange("a b -> (a b)").rearrange(
                "(j p) -> p j", p=128)
            nc.gpsimd.dma_start(counts_g[:], cc_cnt[:, half * 4:(half + 1) * 4])
            ceps = ema.tile([128, 4], f32, tag="ce", name="ce")
            nc.vector.tensor_scalar_add(ceps[:], counts_g[:], 1e-6)
            crec = ema.tile([128, 4], f32, tag="cr", name="cr")
            nc.vector.reciprocal(crec[:], ceps[:])
            for jj in range(4):
                j = half * 4 + jj
                eg = ema.tile([128, 512], f16, tag="eg", name="eg")
                nc.gpsimd.dma_start(eg[:], ccout[half][jj * 128:(jj + 1) * 128, :])
                mj = ema.tile([128, 512], f32, tag="mj2", name="mj2")
                nc.gpsimd.dma_start(mj[:], m[j * 128:(j + 1) * 128, :])
                em = ema.tile([128, 512], f32, tag="em", name="em")
                nc.vector.tensor_scalar(em[:], eg[:], crec[:, jj:jj + 1], 1.0 - RATE,
                                        ALU.mult, op1=ALU.mult)
                nm = ema.tile([128, 512], f32, tag="nm", name="nm")
                nc.vector.scalar_tensor_tensor(nm[:], mj[:], RATE, em[:],
                                               ALU.mult, ALU.add)
                mn2j = _normalize_rows(nc, ema, nm, "e")
                for cc in range(4):
                    pt = psT.tile([128, 128], f32, tag="pt", name="pt")
                    nc.tensor.transpose(pt[:], mn2j[:, cc * 128:(cc + 1) * 128],
                                        ident[:])
                    nc.vector.tensor_copy(mn2T[cc][:, j * 128:(j + 1) * 128], pt[:])
            if half == 0:
                # head-start: first HS subtiles' k-lower raw2 while the second
                # collective half is still in flight
                for t in range(HS):
                    off = t * 128
                    ph = psC.tile([128, 512], f32, tag="raw2", name="raw2")
                    for cc in range(4):
                        nc.tensor.matmul(ph[:], xb16[cc][:, off:off + 128],
                                         mn2T[cc][:, 0:512],
                                         start=(cc == 0), stop=(cc == 3))
                    sh = s2h_pool.tile([128, 512], f32, tag="s2h", name=f"s2h{t}")
                    nc.vector.tensor_scalar_mul(sh[:], ph[:], norms[:, t:t + 1])
                    s2h.append(sh)

    # ---- phase C: score2, softmax, out ----
    with tc.tile_pool(name="wc", bufs=3) as wc:
        for t in range(NT):
            off = t * 128
            halves = []
            for kb in range(2):
                if kb == 0 and t < HS:
                    halves.append(s2h[t])
                    continue
                p = psC.tile([128, 512], f32, tag="raw2", name="raw2")
                for cc in range(4):
                    nc.tensor.matmul(
                        p[:], xb16[cc][:, off:off + 128],
                        mn2T[cc][:, kb * 512:(kb + 1) * 512],
                        start=(cc == 0), stop=(cc == 3))
                sv = wc.tile([128, 512], f32, tag=f"s2{kb}", name=f"s2{kb}")
                nc.vector.tensor_scalar_mul(sv[:], p[:], norms[:, t:t + 1])
                halves.append(sv)
            e_sb = wc.tile([128, K], f32, tag="esb", name="esb")
            zk = wc.tile([128, 2], f32, tag="zk", name="zk")
            for kb in range(2):
                nc.gpsimd.dma_start(
                    score2_o[t * 128:(t + 1) * 128, kb * 512:(kb + 1) * 512],
                    halves[kb][:])
                nc.scalar.activation(
                    e_sb[:, kb * 512:(kb + 1) * 512], halves[kb][:], AF.Exp,
                    accum_out=zk[:, kb:kb + 1])
            z = wc.tile([128, 1], f32, tag="z", name="z")
            nc.vector.tensor_add(z[:], zk[:, 0:1], zk[:, 1:2])
            zr = wc.tile([128, 1], f32, tag="zr", name="zr")
            nc.vector.reciprocal(zr[:], z[:])
            et = wc.tile([128, K], f16, tag="et", name="et")
            for j in range(8):
                pt = psT.tile([128, 128], f32, tag="pt", name="pt")
                nc.tensor.transpose(pt[:], e_sb[:, j * 128:(j + 1) * 128], ident[:])
                nc.vector.tensor_copy(et[:, j * 128:(j + 1) * 128], pt[:])
            pu = psU.tile([128, 512], f32, tag="pu", name="pu")
            for j in range(8):
                nc.tensor.matmul(pu[:], et[:, j * 128:(j + 1) * 128],
                                 std16[:, j * 512:(j + 1) * 512],
                                 start=(j == 0), stop=(j == 7))
            uo = wc.tile([128, 512], f32, tag="uo", name="uo")
            nc.vector.tensor_scalar_mul(uo[:], pu[:], zr[:])
            nc.gpsimd.dma_start(outv_o[t * 128:(t + 1) * 128, :], uo[:])


_NC_CACHE = None


def _build():
    global _NC_CACHE
    if _NC_CACHE is not None:
        return _NC_CACHE
    nc = bacc.Bacc("TRN2", target_bir_lowering=False, debug=False,
                   enable_asserts=True, num_devices=NCORES)
    x16 = nc.dram_tensor("x16", [C, NLOC], f16, kind="ExternalInput").ap()
    m = nc.dram_tensor("m", [K, C], f32, kind="ExternalInput").ap()
    stdt = nc.dram_tensor("stdt", [K, V], f32, kind="ExternalInput").ap()
    ident = nc.dram_tensor("ident", [128, 128], f32, kind="ExternalInput").ap()
    score2_o = nc.dram_tensor("score2", [NLOC, K], f32, kind="ExternalOutput").ap()
    outv_o = nc.dram_tensor("outv", [NLOC, V], f32, kind="ExternalOutput").ap()
    with tile.TileContext(nc, trace_sim=False) as tc, ExitStack() as ctx:
        _kernel(ctx, tc, x16, m, stdt, ident, score2_o, outv_o)
    nc.compile()
    _NC_CACHE = nc
    return nc


def _in_maps(x, noise_feature, std):
    return [
        {
            "x16": np.ascontiguousarray(
                x[i].reshape(C, NLOC)).astype(np.float16),
            "m": np.asarray(noise_feature, dtype=np.float32),
            "stdt": np.asarray(std, dtype=np.float32),
            "ident": np.eye(128, dtype=np.float32),
        }
        for i in range(NCORES)
    ]


def _gather(results):
    score2 = np.concatenate([r["score2"] for r in results], axis=0)
    out = np.stack([r["outv"].reshape(H, W, V) for r in results], axis=0)
    return out, score2


def kernel(x, noise_feature, std):
    nc = _build()
    res = run_bass_kernel_spmd(
        nc, _in_maps(np.asarray(x), np.asarray(noise_feature), np.asarray(std)),
        core_ids=list(range(NCORES)))
    return _gather(res.results)
